# revision 15
# baseline (speedup 1.0000x reference)
"""GQA attention (B=1, T=2048, D=2048, H=32, KVH=8, HD=64) on 8 TRN2 cores.

Head-tensor-parallel: core c owns kv-head c and q-heads 4c..4c+3.
wq/wk/wv column-parallel, wo row-parallel; partials summed on host.

v3: consts stream on the gpsimd DMA queue ahead of the xt flood, q/kv
projections share the DMA-bound k-loop, merged KV psum tile with wide
rope ops, exact-causal diagonal tiles (query-restricted) with a single
[128,128]-triangle mask multiply, per-half-score exp pipelining, and a
4-bank output-projection pool that starts without waiting on the last
softmax normalization.
"""
import sys

if "/opt/trn_rl_repo" not in sys.path:
    sys.path.insert(0, "/opt/trn_rl_repo")

import numpy as np
import ml_dtypes

import concourse.bacc as bacc
import concourse.mybir as mybir
import concourse.tile as tile
from concourse.bass_utils import run_bass_kernel_spmd

BF16 = ml_dtypes.bfloat16
T, D, H, KVH, HD = 2048, 2048, 32, 8, 64
NCORES = 8
HPC = H // NCORES            # 4 q heads per core
KT, PT = 16, 128             # k-tiles of 128 over D
NCH = 4                      # t chunks of 512
CH = 512

_cache = {}


def _build_nc():
    if "nc" in _cache:
        return _cache["nc"]
    fp32, bf16 = mybir.dt.float32, mybir.dt.bfloat16
    Exp = mybir.ActivationFunctionType.Exp
    mult = mybir.AluOpType.mult
    nc = bacc.Bacc("TRN2", target_bir_lowering=False, debug=False,
                   num_devices=NCORES)

    xt_d = nc.dram_tensor("xt", [D, T], bf16, kind="ExternalInput")
    wq_d = nc.dram_tensor("wq", [D, HPC * HD], bf16, kind="ExternalInput")
    wkv_d = nc.dram_tensor("wkv", [D, 2 * HD], bf16, kind="ExternalInput")
    wo_d = nc.dram_tensor("wo", [HPC * HD, D], bf16, kind="ExternalInput")
    cs4_d = nc.dram_tensor("cs4", [32, T], bf16, kind="ExternalInput")
    sn4_d = nc.dram_tensor("sn4", [32, T], bf16, kind="ExternalInput")
    id_d = nc.dram_tensor("ident", [PT, PT], bf16, kind="ExternalInput")
    mk_d = nc.dram_tensor("mask1", [PT, 2, PT], bf16, kind="ExternalInput")
    out_d = nc.dram_tensor("partial", [T, D], bf16, kind="ExternalOutput")

    with tile.TileContext(nc) as tc:
        with tc.tile_pool(name="const", bufs=1) as const, \
             tc.tile_pool(name="xtp", bufs=1) as xtp, \
             tc.tile_pool(name="persist", bufs=1) as persist:

            # ---- loads: wkv + consts go first (gpsimd queue is idle),
            # xt/wq stream splits across the sync and scalar queues
            wkv_sb = const.tile([PT, KT, 2 * HD], bf16, tag="wkv")
            nc.sync.dma_start(wkv_sb[:], wkv_d.ap().rearrange("(k p) m -> p k m", p=PT))
            cs4 = const.tile([PT, NCH, CH], bf16, tag="cs4")
            sn4 = const.tile([PT, NCH, CH], bf16, tag="sn4")
            ident = const.tile([PT, PT], bf16, tag="ident")
            mask1 = const.tile([PT, 2, PT], bf16, tag="mask1")
            wq_sb = const.tile([PT, KT, HPC * HD], bf16, tag="wq")
            wo_sb = const.tile([PT, 2, D], bf16, tag="wo")
            dummy = const.tile([1, 2], bf16, tag="dummy")
            # xt in four 2MB group tiles, two per queue, to amortize the
            # per-DMA descriptor/semaphore overhead on the queues
            nc.scalar.dma_start(wq_sb[:], wq_d.ap().rearrange("(k p) m -> p k m", p=PT))
            xt4 = []
            for g in range(4):
                t_ = xtp.tile([PT, 4, T], bf16, tag=f"xt4_{g}", name=f"xt4_{g}")
                eng = nc.sync if g % 2 == 0 else nc.scalar
                eng.dma_start(t_[:], xt_d.ap()[4 * g * PT:4 * (g + 1) * PT, :]
                              .rearrange("(k p) t -> p k t", p=PT))
                xt4.append(t_)
            xt = [xt4[k // 4][:, k % 4, :] for k in range(KT)]
            c30 = cs4.rearrange("p j c -> p (j c)")
            s30 = sn4.rearrange("p j c -> p (j c)")
            nc.gpsimd.dma_start(c30[0:32, :], cs4_d.ap())
            nc.gpsimd.dma_start(s30[0:32, :], sn4_d.ap())
            nc.gpsimd.dma_start(ident[:], id_d.ap())
            nc.gpsimd.dma_start(mask1[:], mk_d.ap())
            for rr in range(1, 4):
                nc.vector.tensor_copy(c30[32 * rr:32 * rr + 32, :], c30[0:32, :])
                nc.vector.tensor_copy(s30[32 * rr:32 * rr + 32, :], s30[0:32, :])
            # wo only needed in phase D: queue it behind the xt stream
            nc.gpsimd.tensor_copy(dummy[:], xt4[3][0:1, 0, 0:2])
            nc.gpsimd.dma_start(wo_sb[:], wo_d.ap().rearrange("(s p) m -> p s m", p=PT))

            # persistent activations: qtc[j] = [h0|h1|h2|h3] qT for chunk j
            qtc = [persist.tile([64, HPC * CH], bf16, tag=f"qtc{j}", name=f"qtc{j}")
                   for j in range(NCH)]
            kt4 = persist.tile([64, NCH, CH], bf16, tag="kt4")
            vt4 = persist.tile([64, NCH, CH], bf16, tag="vt4")
            vx = [persist.tile([PT, HD + 1], bf16, tag=f"vx{s}", name=f"vx{s}")
                  for s in range(KT)]
            ot = [persist.tile([PT, T], bf16, tag=f"ot{p}", name=f"ot{p}")
                  for p in range(2)]

            def rope_q(tmp, E, O, j):
                """E/O PSUM [128,512] (4 heads x 32 rows) -> qtc[j]."""
                t1 = tmp.tile([PT, CH], bf16, tag="t1")
                t2 = tmp.tile([PT, CH], bf16, tag="t2")
                t3 = tmp.tile([PT, CH], bf16, tag="t3")
                t4 = tmp.tile([PT, CH], bf16, tag="t4")
                nc.vector.tensor_tensor(t1[:], E[:], cs4[:, j, :], mult)
                nc.vector.tensor_tensor(t2[:], O[:], sn4[:, j, :], mult)
                nc.vector.tensor_tensor(t3[:], E[:], sn4[:, j, :], mult)
                nc.vector.tensor_tensor(t4[:], O[:], cs4[:, j, :], mult)
                for h in range(HPC):
                    hsl = slice(h * CH, (h + 1) * CH)
                    rsl = slice(32 * h, 32 * h + 32)
                    nc.vector.tensor_sub(qtc[j][0:32, hsl], t1[rsl, :], t2[rsl, :])
                    nc.vector.tensor_add(qtc[j][32:64, hsl], t3[rsl, :], t4[rsl, :])

            # ---- phase A: kv + q(j=0,1) projections, paced to xt arrivals
            with tc.tile_pool(name="kvp", bufs=1, space="PSUM") as kvp, \
                 tc.tile_pool(name="qe01", bufs=1, space="PSUM") as qe01, \
                 tc.tile_pool(name="tmpa", bufs=2) as tmpa:
                KV4 = kvp.tile([PT, NCH, CH], fp32, tag="kv4")
                EO = [qe01.tile([PT, CH], fp32, tag=f"eo{n}", name=f"eo{n}")
                      for n in range(4)]  # E0, O0, E1, O1
                for k in range(KT):
                    st, sp = (k == 0), (k == KT - 1)
                    for j in range(NCH):
                        nc.tensor.matmul(KV4[:, j, :], wkv_sb[:, k, :],
                                         xt[k][:, j * CH:(j + 1) * CH],
                                         start=st, stop=sp)
                    for j in range(2):
                        jsl = slice(j * CH, (j + 1) * CH)
                        nc.tensor.matmul(EO[2 * j][:], wq_sb[:, k, 0:PT],
                                         xt[k][:, jsl], start=st, stop=sp)
                        nc.tensor.matmul(EO[2 * j + 1][:], wq_sb[:, k, PT:2 * PT],
                                         xt[k][:, jsl], start=st, stop=sp)
                # q rope for chunks 0,1 first (C waits on qtc, not kt)
                rope_q(tmpa, EO[0], EO[1], 0)
                rope_q(tmpa, EO[2], EO[3], 1)
                # rope K for all chunks in wide ops + stash V
                k1 = tmpa.tile([32, NCH, CH], bf16, tag="k1")
                k2 = tmpa.tile([32, NCH, CH], bf16, tag="k2")
                nc.vector.tensor_tensor(k1[:], KV4[0:32, :, :], cs4[0:32, :, :], mult)
                nc.vector.tensor_tensor(k2[:], KV4[32:64, :, :], sn4[0:32, :, :], mult)
                nc.vector.tensor_sub(kt4[0:32, :, :], k1[:], k2[:])
                k3 = tmpa.tile([32, NCH, CH], bf16, tag="k1")
                k4 = tmpa.tile([32, NCH, CH], bf16, tag="k2")
                nc.vector.tensor_tensor(k3[:], KV4[0:32, :, :], sn4[0:32, :, :], mult)
                nc.vector.tensor_tensor(k4[:], KV4[32:64, :, :], cs4[0:32, :, :], mult)
                nc.vector.tensor_add(kt4[32:64, :, :], k3[:], k4[:])
                nc.vector.tensor_copy(vt4[:], KV4[64:PT, :, :])

            # ---- phase B: V transposes + q(j=2,3) projections
            with tc.tile_pool(name="qe23", bufs=1, space="PSUM") as qe23, \
                 tc.tile_pool(name="vtrp", bufs=2, space="PSUM") as vtrp, \
                 tc.tile_pool(name="tmpb", bufs=2) as tmpb:
                EO2 = [qe23.tile([PT, CH], fp32, tag=f"eo2{n}", name=f"eo2{n}")
                       for n in range(4)]  # E2, O2, E3, O3
                for k in range(KT):
                    st, sp = (k == 0), (k == KT - 1)
                    for j in range(2, NCH):
                        jsl = slice(j * CH, (j + 1) * CH)
                        nc.tensor.matmul(EO2[2 * (j - 2)][:], wq_sb[:, k, 0:PT],
                                         xt[k][:, jsl], start=st, stop=sp)
                        nc.tensor.matmul(EO2[2 * (j - 2) + 1][:], wq_sb[:, k, PT:2 * PT],
                                         xt[k][:, jsl], start=st, stop=sp)
                for j in range(NCH):
                    for u in range(4):
                        s_idx = 4 * j + u
                        vtr = vtrp.tile([PT, 64], bf16, tag="vtr")
                        nc.tensor.transpose(vtr[:], vt4[:, j, u * PT:(u + 1) * PT],
                                            ident[:64, :64])
                        nc.vector.tensor_copy(vx[s_idx][:, 0:HD], vtr[:])
                        nc.vector.memset(vx[s_idx][:, HD:HD + 1], 1.0)
                rope_q(tmpb, EO2[0], EO2[1], 2)
                rope_q(tmpb, EO2[2], EO2[3], 3)

            # ---- phase C: attention; per-half-score exp pipelining ----
            with tc.tile_pool(name="sc", bufs=2, space="PSUM") as scp, \
                 tc.tile_pool(name="pv", bufs=1, space="PSUM") as pvp, \
                 tc.tile_pool(name="ex", bufs=4) as exp_pool, \
                 tc.tile_pool(name="nrm", bufs=2) as nrm:
                for j in range(NCH):
                    pv = [pvp.tile([HD + 1, 2, CH], fp32, tag=f"pv{g}", name=f"pv{g}_{j}")
                          for g in range(2)]
                    ni = 4 * j + 4

                    def sc_part(i, j=j):
                        r = i - 4 * j
                        w = CH - 128 * r if r >= 0 else CH
                        q0 = CH - w
                        ktsl = kt4[:, i // 4, (i % 4) * PT:(i % 4) * PT + PT]
                        halves = []
                        for g in range(2):  # head pairs (0,1) and (2,3)
                            sc = scp.tile([PT, 2, CH], fp32, tag="sc")
                            for hh in range(2):
                                h = 2 * g + hh
                                nc.tensor.matmul(
                                    sc[:, hh, 0:w], ktsl,
                                    qtc[j][:, h * CH + q0:(h + 1) * CH],
                                    start=True, stop=True)
                            ex = exp_pool.tile([PT, 2, CH], bf16, tag="ex")
                            nc.scalar.activation(ex[:, :, 0:w], sc[:, :, 0:w],
                                                 Exp, scale=0.125)
                            if r >= 0:
                                # triangle lives only in the first 128 cols
                                nc.vector.tensor_tensor(
                                    ex[:, :, 0:PT], ex[:, :, 0:PT],
                                    mask1[:], mult)
                            halves.append(ex)
                        return halves

                    def pv_part(i, halves, j=j, pv=pv, ni=ni):
                        r = i - 4 * j
                        w = CH - 128 * r if r >= 0 else CH
                        q0 = CH - w
                        for g in range(2):
                            for hh in range(2):
                                nc.tensor.matmul(
                                    pv[g][:, hh, q0:CH], vx[i],
                                    halves[g][:, hh, 0:w],
                                    start=(i == 0), stop=(i == ni - 1),
                                    skip_group_check=True)

                    # emit the first two score groups before any pv matmul so
                    # the PE has work while the previous chunk normalizes
                    h0 = sc_part(0)
                    h1 = sc_part(1)
                    pv_part(0, h0)
                    pv_part(1, h1)
                    for i in range(2, ni):
                        pv_part(i, sc_part(i))
                    # normalize: ot rows = pv[g][0:64, hh] / pv[g][64, hh]
                    for g in range(2):
                        srow = nrm.tile([1, 2, CH], fp32, tag="srow")
                        nc.vector.tensor_copy(srow[:], pv[g][HD:HD + 1, :, :])
                        rrow = nrm.tile([1, 2, CH], fp32, tag="rrow")
                        nc.vector.reciprocal_approx_fast(rrow[:], srow[:])
                        bc = nrm.tile([64, 2, CH], fp32, tag="bc")
                        nc.gpsimd.partition_broadcast(bc[:], rrow[:])
                        for hh in range(2):
                            nc.vector.tensor_tensor(
                                ot[g][64 * hh:64 * hh + 64,
                                      j * CH:(j + 1) * CH],
                                pv[g][0:HD, hh, :], bc[:, hh, :], mult)

            # ---- phase D: output projection ----
            with tc.tile_pool(name="wp", bufs=1, space="PSUM") as wpp, \
                 tc.tile_pool(name="po", bufs=6) as pop:
                wps = [wpp.tile([PT, 2, CH], fp32, tag=f"wp{dd}", name=f"wp{dd}")
                       for dd in range(2)]
                n = 0
                for tt in range(KT):
                    for s in range(2):
                        for dd in range(2):
                            for q in range(2):
                                nc.tensor.matmul(
                                    wps[dd][:, q, :],
                                    ot[s][:, tt * PT:(tt + 1) * PT],
                                    wo_sb[:, s, (2 * dd + q) * CH:(2 * dd + q + 1) * CH],
                                    start=(s == 0), stop=(s == 1))
                    for dd in range(2):
                        pout = pop.tile([PT, 2 * CH], bf16, tag="po")
                        if n % 2 == 0:
                            nc.scalar.copy(pout[:], wps[dd][:])
                        else:
                            nc.vector.tensor_copy(pout[:], wps[dd][:])
                        eng = nc.sync if n % 2 == 0 else nc.gpsimd
                        n += 1
                        eng.dma_start(
                            out_d.ap()[tt * PT:(tt + 1) * PT,
                                       2 * dd * CH:2 * (dd + 1) * CH],
                            pout[:])

    nc.compile()
    _cache["nc"] = nc
    return nc


def _host_prep(x, freqs, wq, wk, wv, wo):
    x2d = np.asarray(x, np.float32)[0]                    # [T, D]
    xt = np.ascontiguousarray(x2d.T).astype(BF16)         # [D, T]
    cos = np.cos(np.asarray(freqs, np.float32))           # [T, 32]
    sin = np.sin(np.asarray(freqs, np.float32))
    cs4 = np.ascontiguousarray(cos.T)                     # [32, T]
    sn4 = np.ascontiguousarray(sin.T)

    ev, od = np.arange(0, HD, 2), np.arange(1, HD, 2)
    ident = np.eye(PT, dtype=np.float32)
    m1 = (np.arange(PT)[None, :] >= np.arange(PT)[:, None]).astype(np.float32)
    mask1 = np.ascontiguousarray(np.broadcast_to(m1[:, None, :], (PT, 2, PT)))

    wq_f = np.asarray(wq, np.float32)
    wk_f = np.asarray(wk, np.float32)
    wv_f = np.asarray(wv, np.float32)
    wo_f = np.asarray(wo, np.float32)

    in_maps = []
    for c in range(NCORES):
        # wq for 4 heads, evens-major-across-heads packing:
        # cols 0:128 = [h0 evens, h1 evens, h2 evens, h3 evens], 128:256 odds
        blocks = [wq_f[:, (c * HPC + h) * HD:(c * HPC + h + 1) * HD] for h in range(HPC)]
        wq_c = np.concatenate([b[:, ev] for b in blocks] + [b[:, od] for b in blocks], axis=1)
        kblk = wk_f[:, c * HD:(c + 1) * HD]
        wkv_c = np.concatenate([kblk[:, ev], kblk[:, od],
                                wv_f[:, c * HD:(c + 1) * HD]], axis=1)
        wo_c = wo_f[c * HPC * HD:(c + 1) * HPC * HD, :]
        in_maps.append({
            "xt": xt,
            "wq": np.ascontiguousarray(wq_c).astype(BF16),
            "wkv": np.ascontiguousarray(wkv_c).astype(BF16),
            "wo": np.ascontiguousarray(wo_c).astype(BF16),
            "cs4": cs4.astype(BF16),
            "sn4": sn4.astype(BF16),
            "ident": ident.astype(BF16),
            "mask1": mask1.astype(BF16),
        })
    return in_maps


def run(inputs, trace=False, tmpdir=None):
    nc = _build_nc()
    in_maps = _host_prep(**inputs)
    res = run_bass_kernel_spmd(nc, in_maps, list(range(NCORES)),
                               trace=trace, tmpdir=tmpdir)
    acc = np.zeros((T, D), np.float32)
    for c in range(NCORES):
        acc += res.results[c]["partial"].astype(np.float32)
    return acc[None], res


def kernel(**inputs):
    out, _ = run(inputs, trace=False)
    return out


# revision 17
# speedup vs baseline: 1.0359x; 1.0359x over previous
"""GQA attention (B=1, T=2048, D=2048, H=32, KVH=8, HD=64) on 8 TRN2 cores.

Head-tensor-parallel: core c owns kv-head c and q-heads 4c..4c+3.
wq/wk/wv column-parallel, wo row-parallel; partials summed on host.

v3: consts stream on the gpsimd DMA queue ahead of the xt flood, q/kv
projections share the DMA-bound k-loop, merged KV psum tile with wide
rope ops, exact-causal diagonal tiles (query-restricted) with a single
[128,128]-triangle mask multiply, per-half-score exp pipelining, and a
4-bank output-projection pool that starts without waiting on the last
softmax normalization.
"""
import sys

if "/opt/trn_rl_repo" not in sys.path:
    sys.path.insert(0, "/opt/trn_rl_repo")

import numpy as np
import ml_dtypes

import concourse.bacc as bacc
import concourse.mybir as mybir
import concourse.tile as tile
from concourse.bass_utils import run_bass_kernel_spmd

BF16 = ml_dtypes.bfloat16
T, D, H, KVH, HD = 2048, 2048, 32, 8, 64
NCORES = 8
HPC = H // NCORES            # 4 q heads per core
KT, PT = 16, 128             # k-tiles of 128 over D
NCH = 4                      # t chunks of 512
CH = 512

_cache = {}


def _build_nc():
    if "nc" in _cache:
        return _cache["nc"]
    fp32, bf16 = mybir.dt.float32, mybir.dt.bfloat16
    Exp = mybir.ActivationFunctionType.Exp
    mult = mybir.AluOpType.mult
    nc = bacc.Bacc("TRN2", target_bir_lowering=False, debug=False,
                   num_devices=NCORES)

    xt_d = nc.dram_tensor("xt", [D, T], bf16, kind="ExternalInput")
    wq_d = nc.dram_tensor("wq", [D, HPC * HD], bf16, kind="ExternalInput")
    wkv_d = nc.dram_tensor("wkv", [D, 2 * HD], bf16, kind="ExternalInput")
    wo_d = nc.dram_tensor("wo", [HPC * HD, D], bf16, kind="ExternalInput")
    cs4_d = nc.dram_tensor("cs4", [32, T], bf16, kind="ExternalInput")
    sn4_d = nc.dram_tensor("sn4", [32, T], bf16, kind="ExternalInput")
    id_d = nc.dram_tensor("ident", [PT, PT], bf16, kind="ExternalInput")
    mk_d = nc.dram_tensor("mask1", [PT, 2, PT], bf16, kind="ExternalInput")
    out_d = nc.dram_tensor("partial", [T, D], bf16, kind="ExternalOutput")

    with tile.TileContext(nc) as tc:
        with tc.tile_pool(name="const", bufs=1) as const, \
             tc.tile_pool(name="xtp", bufs=1) as xtp, \
             tc.tile_pool(name="persist", bufs=1) as persist:

            # ---- loads: wkv + consts go first (gpsimd queue is idle),
            # xt/wq stream splits across the sync and scalar queues
            wkv_sb = const.tile([PT, KT, 2 * HD], bf16, tag="wkv")
            nc.sync.dma_start(wkv_sb[:], wkv_d.ap().rearrange("(k p) m -> p k m", p=PT))
            cs4 = const.tile([PT, NCH, CH], bf16, tag="cs4")
            sn4 = const.tile([PT, NCH, CH], bf16, tag="sn4")
            ident = const.tile([PT, PT], bf16, tag="ident")
            mask1 = const.tile([PT, 2, PT], bf16, tag="mask1")
            wq_sb = const.tile([PT, KT, HPC * HD], bf16, tag="wq")
            wo_sb = const.tile([PT, 2, D], bf16, tag="wo")
            dummy = const.tile([1, 2], bf16, tag="dummy")
            # xt in 5 group DMAs (first tiny so projections start early),
            # split across the sync and scalar queues
            nc.scalar.dma_start(wq_sb[:], wq_d.ap().rearrange("(k p) m -> p k m", p=PT))
            xgroups = [(0, 1), (1, 3), (4, 4), (8, 4), (12, 4)]
            xt = [None] * KT
            for gi, (k0, nk) in enumerate(xgroups):
                t_ = xtp.tile([PT, nk, T], bf16, tag=f"xt4_{gi}", name=f"xt4_{gi}")
                eng = nc.sync if gi % 2 == 0 else nc.scalar
                eng.dma_start(t_[:], xt_d.ap()[k0 * PT:(k0 + nk) * PT, :]
                              .rearrange("(k p) t -> p k t", p=PT))
                for kk in range(nk):
                    xt[k0 + kk] = t_[:, kk, :]
                if gi == 4:
                    xt_last = t_
            c30 = cs4.rearrange("p j c -> p (j c)")
            s30 = sn4.rearrange("p j c -> p (j c)")
            nc.gpsimd.dma_start(c30[0:32, :], cs4_d.ap())
            nc.gpsimd.dma_start(s30[0:32, :], sn4_d.ap())
            nc.gpsimd.dma_start(ident[:], id_d.ap())
            nc.gpsimd.dma_start(mask1[:], mk_d.ap())
            # tile cos/sin x4 on the idle vector engine during the k-loop
            for rr in range(1, 4):
                nc.vector.tensor_copy(c30[32 * rr:32 * rr + 32, :], c30[0:32, :])
                nc.vector.tensor_copy(s30[32 * rr:32 * rr + 32, :], s30[0:32, :])
            # wo only needed in phase D: queue it behind the xt stream
            nc.gpsimd.tensor_copy(dummy[:], xt_last[0:1, 0, 0:2])
            nc.gpsimd.dma_start(wo_sb[:], wo_d.ap().rearrange("(s p) m -> p s m", p=PT))

            # persistent activations: qtc[j] = [h0|h1|h2|h3] qT for chunk j
            qtc = [persist.tile([64, HPC * CH], bf16, tag=f"qtc{j}", name=f"qtc{j}")
                   for j in range(NCH)]
            kt4 = persist.tile([64, NCH, CH], bf16, tag="kt4")
            vt4 = persist.tile([64, NCH, CH], bf16, tag="vt4")
            vx = [persist.tile([PT, HD + 1], bf16, tag=f"vx{s}", name=f"vx{s}")
                  for s in range(KT)]
            ot = [persist.tile([PT, T], bf16, tag=f"ot{p}", name=f"ot{p}")
                  for p in range(2)]

            def rope_q(tmp, E, O, j):
                """E/O PSUM [128,512] -> scalar-staged bf16 -> qtc[j]."""
                eb = tmp.tile([PT, CH], bf16, tag="eb")
                ob = tmp.tile([PT, CH], bf16, tag="ob")
                nc.scalar.copy(eb[:], E[:])
                nc.scalar.copy(ob[:], O[:])
                t1 = tmp.tile([PT, CH], bf16, tag="t1")
                t2 = tmp.tile([PT, CH], bf16, tag="t2")
                t3 = tmp.tile([PT, CH], bf16, tag="t3")
                t4 = tmp.tile([PT, CH], bf16, tag="t4")
                nc.vector.tensor_tensor(t1[:], eb[:], cs4[:, j, :], mult)
                nc.vector.tensor_tensor(t2[:], ob[:], sn4[:, j, :], mult)
                nc.vector.tensor_tensor(t3[:], eb[:], sn4[:, j, :], mult)
                nc.vector.tensor_tensor(t4[:], ob[:], cs4[:, j, :], mult)
                for h in range(HPC):
                    hsl = slice(h * CH, (h + 1) * CH)
                    rsl = slice(32 * h, 32 * h + 32)
                    nc.vector.tensor_sub(qtc[j][0:32, hsl], t1[rsl, :], t2[rsl, :])
                    nc.vector.tensor_add(qtc[j][32:64, hsl], t3[rsl, :], t4[rsl, :])

            # ---- phase A+B: projections; EO23 reuses the EO01 banks as
            # soon as the scalar staging copies have read them
            with tc.tile_pool(name="qe", bufs=1, space="PSUM") as qe, \
                 tc.tile_pool(name="tmpa", bufs=2) as tmpa:
                with tc.tile_pool(name="kvp", bufs=1, space="PSUM") as kvp:
                    KV4 = kvp.tile([PT, NCH, CH], fp32, tag="kv4")
                    EO = [qe.tile([PT, CH], fp32, tag=f"eo{n}", name=f"eo{n}")
                          for n in range(4)]  # E0, O0, E1, O1
                    for k in range(KT):
                        st, sp = (k == 0), (k == KT - 1)
                        for j in range(NCH):
                            nc.tensor.matmul(KV4[:, j, :], wkv_sb[:, k, :],
                                             xt[k][:, j * CH:(j + 1) * CH],
                                             start=st, stop=sp)
                        for j in range(2):
                            jsl = slice(j * CH, (j + 1) * CH)
                            nc.tensor.matmul(EO[2 * j][:], wq_sb[:, k, 0:PT],
                                             xt[k][:, jsl], start=st, stop=sp)
                            nc.tensor.matmul(EO[2 * j + 1][:], wq_sb[:, k, PT:2 * PT],
                                             xt[k][:, jsl], start=st, stop=sp)
                    rope_q(tmpa, EO[0], EO[1], 0)
                    rope_q(tmpa, EO[2], EO[3], 1)
                    # q(j=2,3) projections into the same banks
                    EO2 = [qe.tile([PT, CH], fp32, tag=f"eo{n}", name=f"eo2{n}")
                           for n in range(4)]  # E2, O2, E3, O3
                    for k in range(KT):
                        st, sp = (k == 0), (k == KT - 1)
                        for j in range(2, NCH):
                            jsl = slice(j * CH, (j + 1) * CH)
                            nc.tensor.matmul(EO2[2 * (j - 2)][:], wq_sb[:, k, 0:PT],
                                             xt[k][:, jsl], start=st, stop=sp)
                            nc.tensor.matmul(EO2[2 * (j - 2) + 1][:], wq_sb[:, k, PT:2 * PT],
                                             xt[k][:, jsl], start=st, stop=sp)
                    # rope K: scalar stages K/V rows to SBUF, vector does 4x math
                    kev = tmpa.tile([32, NCH, CH], bf16, tag="kev")
                    kod = tmpa.tile([32, NCH, CH], bf16, tag="kod")
                    nc.scalar.copy(kev[:], KV4[0:32, :, :])
                    nc.scalar.copy(kod[:], KV4[32:64, :, :])
                    nc.scalar.copy(vt4[:], KV4[64:PT, :, :])
                    k1 = tmpa.tile([32, NCH, CH], bf16, tag="k1")
                    k2 = tmpa.tile([32, NCH, CH], bf16, tag="k2")
                    nc.vector.tensor_tensor(k1[:], kev[:], cs4[0:32, :, :], mult)
                    nc.vector.tensor_tensor(k2[:], kod[:], sn4[0:32, :, :], mult)
                    nc.vector.tensor_sub(kt4[0:32, :, :], k1[:], k2[:])
                    k3 = tmpa.tile([32, NCH, CH], bf16, tag="k1")
                    k4 = tmpa.tile([32, NCH, CH], bf16, tag="k2")
                    nc.vector.tensor_tensor(k3[:], kev[:], sn4[0:32, :, :], mult)
                    nc.vector.tensor_tensor(k4[:], kod[:], cs4[0:32, :, :], mult)
                    nc.vector.tensor_add(kt4[32:64, :, :], k3[:], k4[:])
                with tc.tile_pool(name="vtrp", bufs=2, space="PSUM") as vtrp:
                    for j in range(NCH):
                        for u in range(4):
                            s_idx = 4 * j + u
                            vtr = vtrp.tile([PT, 64], bf16, tag="vtr")
                            nc.tensor.transpose(vtr[:], vt4[:, j, u * PT:(u + 1) * PT],
                                                ident[:64, :64])
                            nc.vector.tensor_copy(vx[s_idx][:, 0:HD], vtr[:])
                            nc.vector.memset(vx[s_idx][:, HD:HD + 1], 1.0)
                    rope_q(tmpa, EO2[0], EO2[1], 2)
                    rope_q(tmpa, EO2[2], EO2[3], 3)

            # ---- phase C: attention; per-half-score exp pipelining ----
            with tc.tile_pool(name="sc", bufs=2, space="PSUM") as scp, \
                 tc.tile_pool(name="pv", bufs=1, space="PSUM") as pvp, \
                 tc.tile_pool(name="ex", bufs=4) as exp_pool, \
                 tc.tile_pool(name="nrm", bufs=2) as nrm:
                for j in range(NCH):
                    pv = [pvp.tile([HD + 1, 2, CH], fp32, tag=f"pv{g}", name=f"pv{g}_{j}")
                          for g in range(2)]
                    ni = 4 * j + 4

                    def sc_part(i, j=j):
                        r = i - 4 * j
                        w = CH - 128 * r if r >= 0 else CH
                        q0 = CH - w
                        ktsl = kt4[:, i // 4, (i % 4) * PT:(i % 4) * PT + PT]
                        halves = []
                        for g in range(2):  # head pairs (0,1) and (2,3)
                            sc = scp.tile([PT, 2, CH], fp32, tag="sc")
                            for hh in range(2):
                                h = 2 * g + hh
                                nc.tensor.matmul(
                                    sc[:, hh, 0:w], ktsl,
                                    qtc[j][:, h * CH + q0:(h + 1) * CH],
                                    start=True, stop=True)
                            ex = exp_pool.tile([PT, 2, CH], bf16, tag="ex")
                            nc.scalar.activation(ex[:, :, 0:w], sc[:, :, 0:w],
                                                 Exp, scale=0.125)
                            if r >= 0:
                                # triangle lives only in the first 128 cols
                                nc.vector.tensor_tensor(
                                    ex[:, :, 0:PT], ex[:, :, 0:PT],
                                    mask1[:], mult)
                            halves.append(ex)
                        return halves

                    def pv_part(i, halves, j=j, pv=pv, ni=ni):
                        r = i - 4 * j
                        w = CH - 128 * r if r >= 0 else CH
                        q0 = CH - w
                        for g in range(2):
                            for hh in range(2):
                                nc.tensor.matmul(
                                    pv[g][:, hh, q0:CH], vx[i],
                                    halves[g][:, hh, 0:w],
                                    start=(i == 0), stop=(i == ni - 1),
                                    skip_group_check=True)

                    # emit the first two score groups before any pv matmul so
                    # the PE has work while the previous chunk normalizes
                    h0 = sc_part(0)
                    h1 = sc_part(1)
                    pv_part(0, h0)
                    pv_part(1, h1)
                    for i in range(2, ni):
                        pv_part(i, sc_part(i))
                    # normalize: ot rows = pv[g][0:64, hh] / pv[g][64, hh]
                    for g in range(2):
                        srow = nrm.tile([1, 2, CH], fp32, tag="srow")
                        nc.vector.tensor_copy(srow[:], pv[g][HD:HD + 1, :, :])
                        rrow = nrm.tile([1, 2, CH], fp32, tag="rrow")
                        nc.vector.reciprocal_approx_fast(rrow[:], srow[:])
                        bc = nrm.tile([64, 2, CH], fp32, tag="bc")
                        nc.gpsimd.partition_broadcast(bc[:], rrow[:])
                        for hh in range(2):
                            nc.vector.tensor_tensor(
                                ot[g][64 * hh:64 * hh + 64,
                                      j * CH:(j + 1) * CH],
                                pv[g][0:HD, hh, :], bc[:, hh, :], mult)

            # ---- phase D: output projection ----
            with tc.tile_pool(name="wp", bufs=1, space="PSUM") as wpp, \
                 tc.tile_pool(name="po", bufs=6) as pop:
                wps = [wpp.tile([PT, 2, CH], fp32, tag=f"wp{dd}", name=f"wp{dd}")
                       for dd in range(2)]
                n = 0
                for tt in range(KT):
                    for s in range(2):
                        for dd in range(2):
                            for q in range(2):
                                nc.tensor.matmul(
                                    wps[dd][:, q, :],
                                    ot[s][:, tt * PT:(tt + 1) * PT],
                                    wo_sb[:, s, (2 * dd + q) * CH:(2 * dd + q + 1) * CH],
                                    start=(s == 0), stop=(s == 1))
                    for dd in range(2):
                        pout = pop.tile([PT, 2 * CH], bf16, tag="po")
                        if n % 2 == 0:
                            nc.scalar.copy(pout[:], wps[dd][:])
                        else:
                            nc.vector.tensor_copy(pout[:], wps[dd][:])
                        eng = nc.sync if n % 2 == 0 else nc.gpsimd
                        n += 1
                        eng.dma_start(
                            out_d.ap()[tt * PT:(tt + 1) * PT,
                                       2 * dd * CH:2 * (dd + 1) * CH],
                            pout[:])

    nc.compile()
    _cache["nc"] = nc
    return nc


def _host_prep(x, freqs, wq, wk, wv, wo):
    x2d = np.asarray(x, np.float32)[0]                    # [T, D]
    xt = np.ascontiguousarray(x2d.T).astype(BF16)         # [D, T]
    cos = np.cos(np.asarray(freqs, np.float32))           # [T, 32]
    sin = np.sin(np.asarray(freqs, np.float32))
    cs4 = np.ascontiguousarray(cos.T)                     # [32, T]
    sn4 = np.ascontiguousarray(sin.T)

    ev, od = np.arange(0, HD, 2), np.arange(1, HD, 2)
    ident = np.eye(PT, dtype=np.float32)
    m1 = (np.arange(PT)[None, :] >= np.arange(PT)[:, None]).astype(np.float32)
    mask1 = np.ascontiguousarray(np.broadcast_to(m1[:, None, :], (PT, 2, PT)))

    wq_f = np.asarray(wq, np.float32)
    wk_f = np.asarray(wk, np.float32)
    wv_f = np.asarray(wv, np.float32)
    wo_f = np.asarray(wo, np.float32)

    in_maps = []
    for c in range(NCORES):
        # wq for 4 heads, evens-major-across-heads packing:
        # cols 0:128 = [h0 evens, h1 evens, h2 evens, h3 evens], 128:256 odds
        blocks = [wq_f[:, (c * HPC + h) * HD:(c * HPC + h + 1) * HD] for h in range(HPC)]
        wq_c = np.concatenate([b[:, ev] for b in blocks] + [b[:, od] for b in blocks], axis=1)
        kblk = wk_f[:, c * HD:(c + 1) * HD]
        wkv_c = np.concatenate([kblk[:, ev], kblk[:, od],
                                wv_f[:, c * HD:(c + 1) * HD]], axis=1)
        wo_c = wo_f[c * HPC * HD:(c + 1) * HPC * HD, :]
        in_maps.append({
            "xt": xt,
            "wq": np.ascontiguousarray(wq_c).astype(BF16),
            "wkv": np.ascontiguousarray(wkv_c).astype(BF16),
            "wo": np.ascontiguousarray(wo_c).astype(BF16),
            "cs4": cs4.astype(BF16),
            "sn4": sn4.astype(BF16),
            "ident": ident.astype(BF16),
            "mask1": mask1.astype(BF16),
        })
    return in_maps


def run(inputs, trace=False, tmpdir=None):
    nc = _build_nc()
    in_maps = _host_prep(**inputs)
    res = run_bass_kernel_spmd(nc, in_maps, list(range(NCORES)),
                               trace=trace, tmpdir=tmpdir)
    acc = np.zeros((T, D), np.float32)
    for c in range(NCORES):
        acc += res.results[c]["partial"].astype(np.float32)
    return acc[None], res


def kernel(**inputs):
    out, _ = run(inputs, trace=False)
    return out


# revision 19
# speedup vs baseline: 1.0386x; 1.0026x over previous
"""GQA attention (B=1, T=2048, D=2048, H=32, KVH=8, HD=64) on 8 TRN2 cores.

Head-tensor-parallel: core c owns kv-head c and q-heads 4c..4c+3.
wq/wk/wv column-parallel, wo row-parallel; partials summed on host.

v3: consts stream on the gpsimd DMA queue ahead of the xt flood, q/kv
projections share the DMA-bound k-loop, merged KV psum tile with wide
rope ops, exact-causal diagonal tiles (query-restricted) with a single
[128,128]-triangle mask multiply, per-half-score exp pipelining, and a
4-bank output-projection pool that starts without waiting on the last
softmax normalization.
"""
import sys

if "/opt/trn_rl_repo" not in sys.path:
    sys.path.insert(0, "/opt/trn_rl_repo")

import numpy as np
import ml_dtypes

import concourse.bacc as bacc
import concourse.mybir as mybir
import concourse.tile as tile
from concourse.bass_utils import run_bass_kernel_spmd

BF16 = ml_dtypes.bfloat16
T, D, H, KVH, HD = 2048, 2048, 32, 8, 64
NCORES = 8
HPC = H // NCORES            # 4 q heads per core
KT, PT = 16, 128             # k-tiles of 128 over D
NCH = 4                      # t chunks of 512
CH = 512

_cache = {}


def _build_nc():
    if "nc" in _cache:
        return _cache["nc"]
    fp32, bf16 = mybir.dt.float32, mybir.dt.bfloat16
    Exp = mybir.ActivationFunctionType.Exp
    mult = mybir.AluOpType.mult
    nc = bacc.Bacc("TRN2", target_bir_lowering=False, debug=False,
                   num_devices=NCORES)

    xt_d = nc.dram_tensor("xt", [D, T], bf16, kind="ExternalInput")
    wq_d = nc.dram_tensor("wq", [D, HPC * HD], bf16, kind="ExternalInput")
    wkv_d = nc.dram_tensor("wkv", [D, 2 * HD], bf16, kind="ExternalInput")
    wo_d = nc.dram_tensor("wo", [HPC * HD, D], bf16, kind="ExternalInput")
    cs4_d = nc.dram_tensor("cs4", [32, T], bf16, kind="ExternalInput")
    sn4_d = nc.dram_tensor("sn4", [32, T], bf16, kind="ExternalInput")
    id_d = nc.dram_tensor("ident", [PT, PT], bf16, kind="ExternalInput")
    mk_d = nc.dram_tensor("mask1", [PT, 2, PT], bf16, kind="ExternalInput")
    out_d = nc.dram_tensor("partial", [T, D], bf16, kind="ExternalOutput")

    with tile.TileContext(nc) as tc:
        with tc.tile_pool(name="const", bufs=1) as const, \
             tc.tile_pool(name="xtp", bufs=1) as xtp, \
             tc.tile_pool(name="persist", bufs=1) as persist:

            # ---- loads: wkv + consts go first (gpsimd queue is idle),
            # xt/wq stream splits across the sync and scalar queues
            wkv_sb = const.tile([PT, KT, 2 * HD], bf16, tag="wkv")
            nc.sync.dma_start(wkv_sb[:], wkv_d.ap().rearrange("(k p) m -> p k m", p=PT))
            cs4 = const.tile([PT, NCH, CH], bf16, tag="cs4")
            sn4 = const.tile([PT, NCH, CH], bf16, tag="sn4")
            ident = const.tile([PT, PT], bf16, tag="ident")
            mask1 = const.tile([PT, 2, PT], bf16, tag="mask1")
            wq_sb = const.tile([PT, KT, HPC * HD], bf16, tag="wq")
            wo_sb = const.tile([PT, 2, D], bf16, tag="wo")
            dummy = const.tile([1, 2], bf16, tag="dummy")
            # xt in 5 group DMAs (first tiny so projections start early),
            # split across the sync and scalar queues
            wq_r = wq_d.ap().rearrange("(k p) m -> p k m", p=PT)
            nc.scalar.dma_start(wq_sb[:, 0, :], wq_r[:, 0, :])
            xgroups = [(0, 1), (1, 3), (4, 4), (8, 4), (12, 4)]
            xt = [None] * KT
            for gi, (k0, nk) in enumerate(xgroups):
                t_ = xtp.tile([PT, nk, T], bf16, tag=f"xt4_{gi}", name=f"xt4_{gi}")
                eng = nc.sync if gi % 2 == 0 else nc.scalar
                eng.dma_start(t_[:], xt_d.ap()[k0 * PT:(k0 + nk) * PT, :]
                              .rearrange("(k p) t -> p k t", p=PT))
                for kk in range(nk):
                    xt[k0 + kk] = t_[:, kk, :]
                if gi == 1:
                    nc.scalar.dma_start(wq_sb[:, 1:KT, :], wq_r[:, 1:KT, :])
                if gi == 4:
                    xt_last = t_
            c30 = cs4.rearrange("p j c -> p (j c)")
            s30 = sn4.rearrange("p j c -> p (j c)")
            nc.gpsimd.dma_start(c30[0:32, :], cs4_d.ap())
            nc.gpsimd.dma_start(s30[0:32, :], sn4_d.ap())
            nc.gpsimd.dma_start(ident[:], id_d.ap())
            nc.gpsimd.dma_start(mask1[:], mk_d.ap())
            # tile cos/sin x4 on the idle vector engine during the k-loop
            for rr in range(1, 4):
                nc.vector.tensor_copy(c30[32 * rr:32 * rr + 32, :], c30[0:32, :])
                nc.vector.tensor_copy(s30[32 * rr:32 * rr + 32, :], s30[0:32, :])
            # wo only needed in phase D: queue it behind the xt stream
            nc.gpsimd.tensor_copy(dummy[:], xt_last[0:1, 0, 0:2])
            nc.gpsimd.dma_start(wo_sb[:], wo_d.ap().rearrange("(s p) m -> p s m", p=PT))

            # persistent activations: qtc[j] = [h0|h1|h2|h3] qT for chunk j
            qtc = [persist.tile([64, HPC * CH], bf16, tag=f"qtc{j}", name=f"qtc{j}")
                   for j in range(NCH)]
            kt4 = persist.tile([64, NCH, CH], bf16, tag="kt4")
            vt4 = persist.tile([64, NCH, CH], bf16, tag="vt4")
            vx = [persist.tile([PT, HD + 1], bf16, tag=f"vx{s}", name=f"vx{s}")
                  for s in range(KT)]
            ot = [persist.tile([PT, T], bf16, tag=f"ot{p}", name=f"ot{p}")
                  for p in range(2)]

            def rope_q(tmp, E, O, j):
                """E/O PSUM [128,512] -> scalar-staged bf16 -> qtc[j]."""
                eb = tmp.tile([PT, CH], bf16, tag="eb")
                ob = tmp.tile([PT, CH], bf16, tag="ob")
                nc.scalar.copy(eb[:], E[:])
                nc.scalar.copy(ob[:], O[:])
                t1 = tmp.tile([PT, CH], bf16, tag="t1")
                t2 = tmp.tile([PT, CH], bf16, tag="t2")
                t3 = tmp.tile([PT, CH], bf16, tag="t3")
                t4 = tmp.tile([PT, CH], bf16, tag="t4")
                nc.vector.tensor_tensor(t1[:], eb[:], cs4[:, j, :], mult)
                nc.vector.tensor_tensor(t2[:], ob[:], sn4[:, j, :], mult)
                nc.vector.tensor_tensor(t3[:], eb[:], sn4[:, j, :], mult)
                nc.vector.tensor_tensor(t4[:], ob[:], cs4[:, j, :], mult)
                for h in range(HPC):
                    hsl = slice(h * CH, (h + 1) * CH)
                    rsl = slice(32 * h, 32 * h + 32)
                    nc.vector.tensor_sub(qtc[j][0:32, hsl], t1[rsl, :], t2[rsl, :])
                    nc.vector.tensor_add(qtc[j][32:64, hsl], t3[rsl, :], t4[rsl, :])

            # ---- phase A+B: projections; EO23 reuses the EO01 banks as
            # soon as the scalar staging copies have read them
            with tc.tile_pool(name="qe", bufs=1, space="PSUM") as qe, \
                 tc.tile_pool(name="tmpa", bufs=2) as tmpa:
                with tc.tile_pool(name="kvp", bufs=1, space="PSUM") as kvp:
                    KV4 = kvp.tile([PT, NCH, CH], fp32, tag="kv4")
                    EO = [qe.tile([PT, CH], fp32, tag=f"eo{n}", name=f"eo{n}")
                          for n in range(4)]  # E0, O0, E1, O1
                    for k in range(KT):
                        st, sp = (k == 0), (k == KT - 1)
                        for j in range(NCH):
                            nc.tensor.matmul(KV4[:, j, :], wkv_sb[:, k, :],
                                             xt[k][:, j * CH:(j + 1) * CH],
                                             start=st, stop=sp)
                        for j in range(2):
                            jsl = slice(j * CH, (j + 1) * CH)
                            nc.tensor.matmul(EO[2 * j][:], wq_sb[:, k, 0:PT],
                                             xt[k][:, jsl], start=st, stop=sp)
                            nc.tensor.matmul(EO[2 * j + 1][:], wq_sb[:, k, PT:2 * PT],
                                             xt[k][:, jsl], start=st, stop=sp)
                    rope_q(tmpa, EO[0], EO[1], 0)
                    rope_q(tmpa, EO[2], EO[3], 1)
                    # q(j=2,3) projections into the same banks
                    EO2 = [qe.tile([PT, CH], fp32, tag=f"eo{n}", name=f"eo2{n}")
                           for n in range(4)]  # E2, O2, E3, O3
                    for k in range(KT):
                        st, sp = (k == 0), (k == KT - 1)
                        for j in range(2, NCH):
                            jsl = slice(j * CH, (j + 1) * CH)
                            nc.tensor.matmul(EO2[2 * (j - 2)][:], wq_sb[:, k, 0:PT],
                                             xt[k][:, jsl], start=st, stop=sp)
                            nc.tensor.matmul(EO2[2 * (j - 2) + 1][:], wq_sb[:, k, PT:2 * PT],
                                             xt[k][:, jsl], start=st, stop=sp)
                    # rope K: scalar stages K/V rows to SBUF, vector does 4x math
                    kev = tmpa.tile([32, NCH, CH], bf16, tag="kev")
                    kod = tmpa.tile([32, NCH, CH], bf16, tag="kod")
                    nc.scalar.copy(kev[:], KV4[0:32, :, :])
                    nc.scalar.copy(kod[:], KV4[32:64, :, :])
                    nc.scalar.copy(vt4[:], KV4[64:PT, :, :])
                    k1 = tmpa.tile([32, NCH, CH], bf16, tag="k1")
                    k2 = tmpa.tile([32, NCH, CH], bf16, tag="k2")
                    nc.vector.tensor_tensor(k1[:], kev[:], cs4[0:32, :, :], mult)
                    nc.vector.tensor_tensor(k2[:], kod[:], sn4[0:32, :, :], mult)
                    nc.vector.tensor_sub(kt4[0:32, :, :], k1[:], k2[:])
                    k3 = tmpa.tile([32, NCH, CH], bf16, tag="k1")
                    k4 = tmpa.tile([32, NCH, CH], bf16, tag="k2")
                    nc.vector.tensor_tensor(k3[:], kev[:], sn4[0:32, :, :], mult)
                    nc.vector.tensor_tensor(k4[:], kod[:], cs4[0:32, :, :], mult)
                    nc.vector.tensor_add(kt4[32:64, :, :], k3[:], k4[:])
                with tc.tile_pool(name="vtrp", bufs=2, space="PSUM") as vtrp:
                    for j in range(NCH):
                        for u in range(4):
                            s_idx = 4 * j + u
                            vtr = vtrp.tile([PT, 64], bf16, tag="vtr")
                            nc.tensor.transpose(vtr[:], vt4[:, j, u * PT:(u + 1) * PT],
                                                ident[:64, :64])
                            nc.vector.tensor_copy(vx[s_idx][:, 0:HD], vtr[:])
                            nc.vector.memset(vx[s_idx][:, HD:HD + 1], 1.0)
                    rope_q(tmpa, EO2[0], EO2[1], 2)
                    rope_q(tmpa, EO2[2], EO2[3], 3)

            # ---- phase C: attention; per-half-score exp pipelining ----
            with tc.tile_pool(name="sc", bufs=2, space="PSUM") as scp, \
                 tc.tile_pool(name="pv", bufs=1, space="PSUM") as pvp, \
                 tc.tile_pool(name="ex", bufs=8) as exp_pool, \
                 tc.tile_pool(name="nrm", bufs=2) as nrm:
                for j in range(NCH):
                    pv = [pvp.tile([HD + 1, 2, CH], fp32, tag=f"pv{g}", name=f"pv{g}_{j}")
                          for g in range(2)]
                    ni = 4 * j + 4

                    def sc_part(i, j=j):
                        r = i - 4 * j
                        w = CH - 128 * r if r >= 0 else CH
                        q0 = CH - w
                        ktsl = kt4[:, i // 4, (i % 4) * PT:(i % 4) * PT + PT]
                        halves = []
                        for g in range(2):  # head pairs (0,1) and (2,3)
                            sc = scp.tile([PT, 2, CH], fp32, tag="sc")
                            for hh in range(2):
                                h = 2 * g + hh
                                nc.tensor.matmul(
                                    sc[:, hh, 0:w], ktsl,
                                    qtc[j][:, h * CH + q0:(h + 1) * CH],
                                    start=True, stop=True)
                            ex = exp_pool.tile([PT, 2, CH], bf16, tag="ex")
                            nc.scalar.activation(ex[:, :, 0:w], sc[:, :, 0:w],
                                                 Exp, scale=0.125)
                            if r >= 0:
                                # triangle lives only in the first 128 cols
                                nc.vector.tensor_tensor(
                                    ex[:, :, 0:PT], ex[:, :, 0:PT],
                                    mask1[:], mult)
                            halves.append(ex)
                        return halves

                    def pv_part(i, halves, j=j, pv=pv, ni=ni):
                        r = i - 4 * j
                        w = CH - 128 * r if r >= 0 else CH
                        q0 = CH - w
                        for g in range(2):
                            for hh in range(2):
                                nc.tensor.matmul(
                                    pv[g][:, hh, q0:CH], vx[i],
                                    halves[g][:, hh, 0:w],
                                    start=(i == 0), stop=(i == ni - 1),
                                    skip_group_check=True)

                    # emit the first four score groups before any pv matmul so
                    # the PE has work while the previous chunk normalizes
                    pre = min(4, ni)
                    hs = [sc_part(i) for i in range(pre)]
                    for i in range(pre):
                        pv_part(i, hs[i])
                    for i in range(pre, ni):
                        pv_part(i, sc_part(i))
                    # normalize: ot rows = pv[g][0:64, hh] / pv[g][64, hh]
                    for g in range(2):
                        srow = nrm.tile([1, 2, CH], fp32, tag="srow")
                        nc.scalar.copy(srow[:], pv[g][HD:HD + 1, :, :])
                        rrow = nrm.tile([1, 2, CH], fp32, tag="rrow")
                        nc.vector.reciprocal_approx_fast(rrow[:], srow[:])
                        bc = nrm.tile([64, 2, CH], fp32, tag="bc")
                        nc.gpsimd.partition_broadcast(bc[:], rrow[:])
                        for hh in range(2):
                            nc.vector.tensor_tensor(
                                ot[g][64 * hh:64 * hh + 64,
                                      j * CH:(j + 1) * CH],
                                pv[g][0:HD, hh, :], bc[:, hh, :], mult)

            # ---- phase D: output projection ----
            with tc.tile_pool(name="wp", bufs=1, space="PSUM") as wpp, \
                 tc.tile_pool(name="po", bufs=6) as pop:
                wps = [wpp.tile([PT, 2, CH], fp32, tag=f"wp{dd}", name=f"wp{dd}")
                       for dd in range(2)]
                n = 0
                for tt in range(KT):
                    for s in range(2):
                        for dd in range(2):
                            for q in range(2):
                                nc.tensor.matmul(
                                    wps[dd][:, q, :],
                                    ot[s][:, tt * PT:(tt + 1) * PT],
                                    wo_sb[:, s, (2 * dd + q) * CH:(2 * dd + q + 1) * CH],
                                    start=(s == 0), stop=(s == 1))
                    for dd in range(2):
                        pout = pop.tile([PT, 2 * CH], bf16, tag="po")
                        if n % 2 == 0:
                            nc.scalar.copy(pout[:], wps[dd][:])
                        else:
                            nc.vector.tensor_copy(pout[:], wps[dd][:])
                        eng = nc.sync if n % 2 == 0 else nc.gpsimd
                        n += 1
                        eng.dma_start(
                            out_d.ap()[tt * PT:(tt + 1) * PT,
                                       2 * dd * CH:2 * (dd + 1) * CH],
                            pout[:])

    nc.compile()
    _cache["nc"] = nc
    return nc


def _host_prep(x, freqs, wq, wk, wv, wo):
    x2d = np.asarray(x, np.float32)[0]                    # [T, D]
    xt = np.ascontiguousarray(x2d.T).astype(BF16)         # [D, T]
    cos = np.cos(np.asarray(freqs, np.float32))           # [T, 32]
    sin = np.sin(np.asarray(freqs, np.float32))
    cs4 = np.ascontiguousarray(cos.T)                     # [32, T]
    sn4 = np.ascontiguousarray(sin.T)

    ev, od = np.arange(0, HD, 2), np.arange(1, HD, 2)
    ident = np.eye(PT, dtype=np.float32)
    m1 = (np.arange(PT)[None, :] >= np.arange(PT)[:, None]).astype(np.float32)
    mask1 = np.ascontiguousarray(np.broadcast_to(m1[:, None, :], (PT, 2, PT)))

    wq_f = np.asarray(wq, np.float32)
    wk_f = np.asarray(wk, np.float32)
    wv_f = np.asarray(wv, np.float32)
    wo_f = np.asarray(wo, np.float32)

    in_maps = []
    for c in range(NCORES):
        # wq for 4 heads, evens-major-across-heads packing:
        # cols 0:128 = [h0 evens, h1 evens, h2 evens, h3 evens], 128:256 odds
        blocks = [wq_f[:, (c * HPC + h) * HD:(c * HPC + h + 1) * HD] for h in range(HPC)]
        wq_c = np.concatenate([b[:, ev] for b in blocks] + [b[:, od] for b in blocks], axis=1)
        kblk = wk_f[:, c * HD:(c + 1) * HD]
        wkv_c = np.concatenate([kblk[:, ev], kblk[:, od],
                                wv_f[:, c * HD:(c + 1) * HD]], axis=1)
        wo_c = wo_f[c * HPC * HD:(c + 1) * HPC * HD, :]
        in_maps.append({
            "xt": xt,
            "wq": np.ascontiguousarray(wq_c).astype(BF16),
            "wkv": np.ascontiguousarray(wkv_c).astype(BF16),
            "wo": np.ascontiguousarray(wo_c).astype(BF16),
            "cs4": cs4.astype(BF16),
            "sn4": sn4.astype(BF16),
            "ident": ident.astype(BF16),
            "mask1": mask1.astype(BF16),
        })
    return in_maps


def run(inputs, trace=False, tmpdir=None):
    nc = _build_nc()
    in_maps = _host_prep(**inputs)
    res = run_bass_kernel_spmd(nc, in_maps, list(range(NCORES)),
                               trace=trace, tmpdir=tmpdir)
    acc = np.zeros((T, D), np.float32)
    for c in range(NCORES):
        acc += res.results[c]["partial"].astype(np.float32)
    return acc[None], res


def kernel(**inputs):
    out, _ = run(inputs, trace=False)
    return out
